# revision 2
# baseline (speedup 1.0000x reference)
"""Bass/Tile TRN2 kernel for nn_IterativeDecimator (gnn_message_passing).

Computes, for a batch of G=256 graphs with P=1024 nodes each (D=128 feats):
  A = softmax(relu(X @ W1 + b1) @ W2 + b2)          # [N, K] cluster assignments
  coarse[g, k, :] = sum_{i in graph g} A[i, k] X[i] # [G*K, D]
plus the (deterministic) fully-connected coarse edge lists.

Sharding: data parallel over graphs — 32 whole graphs per NeuronCore, MLP
weights replicated. No cross-device communication needed.

Per-core device pipeline (per graph g, nodes in 8 tiles of 128):
  DMA X_g -> SBUF [128 part = node-in-tile, 8 tiles, 128 feats]
  PE transpose X tiles -> X^T (feat-major) for the D-contraction
  mm1: h^T[32,512] = W1^T @ X^T     (PSUM), ACT relu+bias -> SBUF
  mm2: logits[128,16] = (h^T slab)^T @ W2 per tile    (PSUM, natural layout)
  softmax along free dim: +b2, -max, ACT exp with fused row-sum, reciprocal, scale
  mm3: C^T[128D,16K] += X_tile^T @ A_tile  accumulated in PSUM over the graph
Outputs are stored partition-major and reassembled on host.
"""

import sys

for _p in ("/opt/trn_rl_repo", "/root/.axon_site/_ro/trn_rl_repo"):
    if _p not in sys.path:
        sys.path.append(_p)

import numpy as np

G, P, D, K, H = 256, 1024, 128, 16, 32
N = G * P
NCORES = 8
GPC = G // NCORES          # graphs per core
NPC = GPC * P              # nodes per core
TPG = P // 128             # 128-node tiles per graph (8)
TILES = NPC // 128         # tiles per core (256)

_prog_cache = {}


def _build_program(mm1_dtype="float32"):
    from contextlib import ExitStack

    import concourse.tile as tile
    from concourse import bacc, mybir

    f32 = mybir.dt.float32
    AF = mybir.ActivationFunctionType

    nc = bacc.Bacc("TRN2", target_bir_lowering=False, debug=False, enable_asserts=True)

    x_d = nc.dram_tensor("x", [NPC, D], f32, kind="ExternalInput")
    w1_d = nc.dram_tensor("w1", [D, H], f32, kind="ExternalInput")
    b1_d = nc.dram_tensor("b1", [H, 1], f32, kind="ExternalInput")
    w2_d = nc.dram_tensor("w2", [H, K], f32, kind="ExternalInput")
    b2r_d = nc.dram_tensor("b2r", [128, 4 * K], f32, kind="ExternalInput")
    id_d = nc.dram_tensor("ident", [128, 128], f32, kind="ExternalInput")

    a_out = nc.dram_tensor("a_out", [128, TILES, K], f32, kind="ExternalOutput")
    c_out = nc.dram_tensor("c_out", [128, GPC * K], f32, kind="ExternalOutput")

    x_t = x_d.ap().rearrange("(n p) d -> p n d", p=128)  # node = n*128 + p

    with tile.TileContext(nc) as tc, ExitStack() as ctx:
        consts = ctx.enter_context(tc.tile_pool(name="consts", bufs=1))
        xg_pool = ctx.enter_context(tc.tile_pool(name="xg", bufs=3))
        xt_pool = ctx.enter_context(tc.tile_pool(name="xt", bufs=2))
        ht_pool = ctx.enter_context(tc.tile_pool(name="ht", bufs=2))
        ag_pool = ctx.enter_context(tc.tile_pool(name="ag", bufs=3))
        lgb_pool = ctx.enter_context(tc.tile_pool(name="lgb", bufs=2))
        e_pool = ctx.enter_context(tc.tile_pool(name="e", bufs=2))
        small = ctx.enter_context(tc.tile_pool(name="small", bufs=2))
        call_pool = ctx.enter_context(tc.tile_pool(name="call", bufs=1))

        xtp_ps = ctx.enter_context(tc.tile_pool(name="xtp", bufs=2, space="PSUM"))
        htp_ps = ctx.enter_context(tc.tile_pool(name="htp", bufs=2, space="PSUM"))
        lgp_ps = ctx.enter_context(tc.tile_pool(name="lgp", bufs=2, space="PSUM"))
        cp_ps = ctx.enter_context(tc.tile_pool(name="cp", bufs=2, space="PSUM"))

        w1 = consts.tile([D, H], f32)
        nc.sync.dma_start(w1[:], w1_d.ap())
        b1 = consts.tile([H, 1], f32)
        nc.sync.dma_start(b1[:], b1_d.ap())
        w2 = consts.tile([H, K], f32)
        nc.sync.dma_start(w2[:], w2_d.ap())
        b2r = consts.tile([128, 4 * K], f32)
        nc.sync.dma_start(b2r[:], b2r_d.ap())
        ident = consts.tile([128, 128], f32)
        nc.sync.dma_start(ident[:], id_d.ap())

        c_all = call_pool.tile([128, GPC * K], f32)

        for g in range(GPC):
            xg = xg_pool.tile([128, TPG, D], f32)
            nc.sync.dma_start(xg[:], x_t[:, g * TPG:(g + 1) * TPG, :])
            ag = ag_pool.tile([128, TPG, K], f32)
            cp = cp_ps.tile([128, K], f32)

            for half in range(2):
                xtp = xtp_ps.tile([128, 512], f32)
                for q in range(4):
                    t = half * 4 + q
                    nc.tensor.transpose(
                        xtp[:, q * 128:(q + 1) * 128], xg[:, t, :], ident[:]
                    )
                xt = xt_pool.tile([128, 512], f32)
                nc.any.tensor_copy(xt[:], xtp[:])

                htp = htp_ps.tile([H, 512], f32)
                nc.tensor.matmul(htp[:], w1[:], xt[:])
                ht = ht_pool.tile([H, 512], f32)
                nc.scalar.activation(ht[:], htp[:], AF.Relu, bias=b1[:])

                lgp = lgp_ps.tile([128, 4 * K], f32)
                for q in range(4):
                    nc.tensor.matmul(
                        lgp[:, q * K:(q + 1) * K],
                        ht[:, q * 128:(q + 1) * 128],
                        w2[:],
                    )

                lgb = lgb_pool.tile([128, 4 * K], f32)
                nc.vector.tensor_add(lgb[:], lgp[:], b2r[:])
                nmx = small.tile([128, 4], f32)
                nc.vector.tensor_reduce(
                    nmx[:],
                    lgb[:].rearrange("p (q k) -> p q k", q=4),
                    axis=mybir.AxisListType.X,
                    op=mybir.AluOpType.max,
                    negate=True,
                )
                e = e_pool.tile([128, 4 * K], f32)
                s = small.tile([128, 4], f32)
                for q in range(4):
                    nc.scalar.activation(
                        e[:, q * K:(q + 1) * K],
                        lgb[:, q * K:(q + 1) * K],
                        AF.Exp,
                        bias=nmx[:, q:q + 1],
                        accum_out=s[:, q:q + 1],
                    )
                r = small.tile([128, 4], f32)
                nc.vector.reciprocal(r[:], s[:])
                for q in range(4):
                    t = half * 4 + q
                    nc.vector.tensor_scalar_mul(
                        ag[:, t, :], e[:, q * K:(q + 1) * K], r[:, q:q + 1]
                    )
                    nc.tensor.matmul(
                        cp[:], xg[:, t, :], ag[:, t, :],
                        start=(t == 0), stop=(t == TPG - 1),
                    )

            nc.any.tensor_copy(c_all[:, g * K:(g + 1) * K], cp[:])
            nc.sync.dma_start(a_out.ap()[:, g * TPG:(g + 1) * TPG, :], ag[:])

        nc.sync.dma_start(c_out.ap(), c_all[:])

    nc.compile()
    return nc


def _get_program():
    if "nc" not in _prog_cache:
        _prog_cache["nc"] = _build_program()
    return _prog_cache["nc"]


def _edges():
    single_senders = np.repeat(np.arange(K), K)
    single_receivers = np.tile(np.arange(K), K)
    batch_offset = np.arange(G)[:, None] * K
    c_senders = (single_senders[None, :] + batch_offset).reshape(-1).astype(np.int32)
    c_receivers = (single_receivers[None, :] + batch_offset).reshape(-1).astype(np.int32)
    c_edges = np.ones((c_senders.shape[0], 1), dtype=np.float32)
    return c_senders, c_receivers, c_edges


def _numpy_fallback(x, W1, b1, W2, b2, n_node):
    h = np.maximum(x @ W1 + b1, 0.0)
    logits = h @ W2 + b2
    lmax = logits.max(axis=-1, keepdims=True)
    ex = np.exp(logits - lmax)
    A = ex / ex.sum(axis=-1, keepdims=True)
    batch = np.repeat(np.arange(n_node.shape[0]), n_node)
    C = np.zeros((n_node.shape[0], K, x.shape[1]), dtype=np.float32)
    for k in range(K):
        np.add.at(C[:, k, :], batch, x * A[:, k:k + 1])
    return C.reshape(-1, x.shape[1]), A


def kernel(node_feats, W1, b1, W2, b2, n_node):
    from concourse import bass_utils

    x = np.ascontiguousarray(np.asarray(node_feats, dtype=np.float32))
    W1 = np.ascontiguousarray(np.asarray(W1, dtype=np.float32))
    b1v = np.asarray(b1, dtype=np.float32).reshape(-1)
    W2 = np.ascontiguousarray(np.asarray(W2, dtype=np.float32))
    b2v = np.asarray(b2, dtype=np.float32).reshape(-1)
    n_node = np.asarray(n_node, dtype=np.int32)

    c_senders, c_receivers, c_edges = _edges()

    if x.shape != (N, D) or not np.all(n_node == P):
        C, A = _numpy_fallback(x, W1, b1v, W2, b2v, n_node)
        return C, A, c_senders, c_receivers, c_edges

    nc = _get_program()

    b1c = b1v.reshape(H, 1).copy()
    b2r = np.tile(b2v.reshape(1, K), (128, 4)).copy()
    ident = np.eye(128, dtype=np.float32)

    in_maps = []
    for c in range(NCORES):
        in_maps.append({
            "x": x[c * NPC:(c + 1) * NPC],
            "w1": W1,
            "b1": b1c,
            "w2": W2,
            "b2r": b2r,
            "ident": ident,
        })

    res = bass_utils.run_bass_kernel_spmd(
        nc, in_maps, core_ids=list(range(NCORES)), trace=False
    )

    A = np.empty((N, K), dtype=np.float32)
    C = np.empty((G * K, D), dtype=np.float32)
    for c in range(NCORES):
        a_buf = res.results[c]["a_out"]              # [128, TILES, K]
        A[c * NPC:(c + 1) * NPC] = a_buf.transpose(1, 0, 2).reshape(NPC, K)
        c_buf = res.results[c]["c_out"].reshape(128, GPC, K)   # [D, g, k]
        C[c * GPC * K:(c + 1) * GPC * K] = (
            c_buf.transpose(1, 2, 0).reshape(GPC * K, D)
        )

    return C, A, c_senders, c_receivers, c_edges


# revision 6
# speedup vs baseline: 84.8643x; 84.8643x over previous
"""Bass/Tile TRN2 kernel for nn_IterativeDecimator (gnn_message_passing).

Computes, for a batch of G=256 graphs with P=1024 nodes each (D=128 feats):
  A = softmax(relu(X @ W1 + b1) @ W2 + b2)          # [N, K] cluster assignments
  coarse[g, k, :] = sum_{i in graph g} A[i, k] X[i] # [G*K, D]
plus the (deterministic) fully-connected coarse edge lists.

Sharding: data parallel over graphs — 32 whole graphs per NeuronCore, MLP
weights replicated. No cross-device communication needed.

Per-core device pipeline (per graph g, nodes in 8 tiles of 128):
  DMA X_g -> SBUF [128 part = node-in-tile, 8 tiles, 128 feats]
  PE transpose X tiles -> X^T (feat-major) for the D-contraction
  mm1: h^T[32,512] = W1^T @ X^T     (PSUM), ACT relu+bias -> SBUF
  mm2: logits[128,16] = (h^T slab)^T @ W2 per tile    (PSUM, natural layout)
  softmax along free dim: +b2, -max, ACT exp with fused row-sum, reciprocal, scale
  mm3: C^T[128D,16K] += X_tile^T @ A_tile  accumulated in PSUM over the graph
Outputs are stored partition-major and reassembled on host.
"""

import sys

for _p in ("/opt/trn_rl_repo", "/root/.axon_site/_ro/trn_rl_repo"):
    if _p not in sys.path:
        sys.path.append(_p)

import numpy as np

G, P, D, K, H = 256, 1024, 128, 16, 32
N = G * P
NCORES = 8
GPC = G // NCORES          # graphs per core
NPC = GPC * P              # nodes per core
TPG = P // 128             # 128-node tiles per graph (8)
TILES = NPC // 128         # tiles per core (256)

_prog_cache = {}


def _build_program(mm1_dtype="float32", loops=1):
    from contextlib import ExitStack

    import concourse.tile as tile
    from concourse import bacc, mybir

    f32 = mybir.dt.float32
    f32r = mybir.dt.float32r
    AF = mybir.ActivationFunctionType
    use_f32r = mm1_dtype == "float32r"

    nc = bacc.Bacc("TRN2", target_bir_lowering=False, debug=False, enable_asserts=True)

    x_d = nc.dram_tensor("x", [NPC, D], f32, kind="ExternalInput")
    w1_d = nc.dram_tensor("w1", [D, H], f32, kind="ExternalInput")
    b1_d = nc.dram_tensor("b1", [H, 1], f32, kind="ExternalInput")
    w2_d = nc.dram_tensor("w2", [H, K], f32, kind="ExternalInput")
    b2r_d = nc.dram_tensor("b2r", [128, 4 * K], f32, kind="ExternalInput")
    id_d = nc.dram_tensor("ident", [128, 128], f32, kind="ExternalInput")

    a_out = nc.dram_tensor("a_out", [128, TILES, K], f32, kind="ExternalOutput")
    c_out = nc.dram_tensor("c_out", [128, GPC * K], f32, kind="ExternalOutput")

    x_t = x_d.ap().rearrange("(n p) d -> p n d", p=128)  # node = n*128 + p

    with tile.TileContext(nc) as tc, ExitStack() as ctx:
        consts = ctx.enter_context(tc.tile_pool(name="consts", bufs=1))
        xg_pool = ctx.enter_context(tc.tile_pool(name="xg", bufs=3))
        xt_pool = ctx.enter_context(tc.tile_pool(name="xt", bufs=2))
        ht_pool = ctx.enter_context(tc.tile_pool(name="ht", bufs=2))
        ag_pool = ctx.enter_context(tc.tile_pool(name="ag", bufs=3))
        lgb_pool = ctx.enter_context(tc.tile_pool(name="lgb", bufs=2))
        e_pool = ctx.enter_context(tc.tile_pool(name="e", bufs=2))
        small = ctx.enter_context(tc.tile_pool(name="small", bufs=2))
        call_pool = ctx.enter_context(tc.tile_pool(name="call", bufs=1))

        xtp_ps = ctx.enter_context(tc.tile_pool(name="xtp", bufs=2, space="PSUM"))
        htp_ps = ctx.enter_context(tc.tile_pool(name="htp", bufs=2, space="PSUM"))
        lgp_ps = ctx.enter_context(tc.tile_pool(name="lgp", bufs=2, space="PSUM"))
        cp_ps = ctx.enter_context(tc.tile_pool(name="cp", bufs=2, space="PSUM"))

        w1 = consts.tile([D, H], f32)
        nc.sync.dma_start(w1[:], w1_d.ap())
        b1 = consts.tile([H, 1], f32)
        nc.sync.dma_start(b1[:], b1_d.ap())
        w2 = consts.tile([H, K], f32)
        nc.sync.dma_start(w2[:], w2_d.ap())
        b2r = consts.tile([128, 4 * K], f32)
        nc.sync.dma_start(b2r[:], b2r_d.ap())
        ident = consts.tile([128, 128], f32)
        nc.sync.dma_start(ident[:], id_d.ap())

        c_all = call_pool.tile([128, GPC * K], f32)

        loop_ctx = tc.For_i(0, loops, 1) if loops > 1 else None
        if loop_ctx is not None:
            ctx.enter_context(loop_ctx)

        for g in range(GPC):
            xg = xg_pool.tile([128, TPG, D], f32)
            nc.sync.dma_start(xg[:], x_t[:, g * TPG:(g + 1) * TPG, :])
            ag = ag_pool.tile([128, TPG, K], f32)
            cp = cp_ps.tile([128, K], f32)

            for half in range(2):
                xtp = xtp_ps.tile([128, 512], f32)
                for q in range(4):
                    t = half * 4 + q
                    nc.tensor.transpose(
                        xtp[:, q * 128:(q + 1) * 128], xg[:, t, :], ident[:]
                    )
                xt = xt_pool.tile([128, 512], f32)
                nc.any.tensor_copy(xt[:], xtp[:])

                htp = htp_ps.tile([H, 512], f32)
                if use_f32r:
                    nc.tensor.matmul(htp[:], w1[:].bitcast(f32r), xt[:].bitcast(f32r))
                else:
                    nc.tensor.matmul(htp[:], w1[:], xt[:])
                ht = ht_pool.tile([H, 512], f32)
                nc.scalar.activation(ht[:], htp[:], AF.Relu, bias=b1[:])

                lgp = lgp_ps.tile([128, 4 * K], f32)
                for q in range(4):
                    nc.tensor.matmul(
                        lgp[:, q * K:(q + 1) * K],
                        ht[:, q * 128:(q + 1) * 128],
                        w2[:],
                    )

                lgb = lgb_pool.tile([128, 4 * K], f32)
                nc.vector.tensor_add(lgb[:], lgp[:], b2r[:])
                nmx = small.tile([128, 4], f32)
                nc.vector.tensor_reduce(
                    nmx[:],
                    lgb[:].rearrange("p (q k) -> p q k", q=4),
                    axis=mybir.AxisListType.X,
                    op=mybir.AluOpType.max,
                    negate=True,
                )
                e = e_pool.tile([128, 4 * K], f32)
                s = small.tile([128, 4], f32)
                for q in range(4):
                    nc.scalar.activation(
                        e[:, q * K:(q + 1) * K],
                        lgb[:, q * K:(q + 1) * K],
                        AF.Exp,
                        bias=nmx[:, q:q + 1],
                        accum_out=s[:, q:q + 1],
                    )
                r = small.tile([128, 4], f32)
                nc.vector.reciprocal(r[:], s[:])
                for q in range(4):
                    t = half * 4 + q
                    nc.vector.tensor_scalar_mul(
                        ag[:, t, :], e[:, q * K:(q + 1) * K], r[:, q:q + 1]
                    )
                    nc.tensor.matmul(
                        cp[:], xg[:, t, :], ag[:, t, :],
                        start=(t == 0), stop=(t == TPG - 1),
                    )

            nc.any.tensor_copy(c_all[:, g * K:(g + 1) * K], cp[:])
            nc.sync.dma_start(a_out.ap()[:, g * TPG:(g + 1) * TPG, :], ag[:])

        nc.sync.dma_start(c_out.ap(), c_all[:])

    nc.compile()
    return nc


import os

MM1_DTYPE = os.environ.get("K_MM1_DTYPE", "float32")


def _get_program(loops=1):
    key = (MM1_DTYPE, loops)
    if key not in _prog_cache:
        _prog_cache[key] = _build_program(mm1_dtype=MM1_DTYPE, loops=loops)
    return _prog_cache[key]


def _make_in_maps(x, W1, b1v, W2, b2v):
    b1c = b1v.reshape(H, 1).copy()
    b2r = np.tile(b2v.reshape(1, K), (128, 4)).copy()
    ident = np.eye(128, dtype=np.float32)
    return [
        {
            "x": x[c * NPC:(c + 1) * NPC],
            "w1": W1, "b1": b1c, "w2": W2, "b2r": b2r, "ident": ident,
        }
        for c in range(NCORES)
    ]


def measure_device_time(node_feats, W1, b1, W2, b2, reps=3, big_loops=9):
    """Differential device-time estimate: wall(loops=R) - wall(loops=1)
    cancels jit/transfer/dispatch overhead. Returns est ns per body pass."""
    import time as _time

    from concourse import bass_utils

    x = np.ascontiguousarray(np.asarray(node_feats, dtype=np.float32))
    in_maps = _make_in_maps(
        x,
        np.ascontiguousarray(np.asarray(W1, dtype=np.float32)),
        np.asarray(b1, dtype=np.float32).reshape(-1),
        np.ascontiguousarray(np.asarray(W2, dtype=np.float32)),
        np.asarray(b2, dtype=np.float32).reshape(-1),
    )
    walls = {}
    for loops in (1, big_loops):
        nc = _get_program(loops)
        ts = []
        for _ in range(reps):
            t0 = _time.time()
            bass_utils.run_bass_kernel_spmd(
                nc, in_maps, core_ids=list(range(NCORES)), trace=False
            )
            ts.append(_time.time() - t0)
        walls[loops] = min(ts)
        print(f"loops={loops}: wall times {[f'{t:.2f}' for t in ts]}", flush=True)
    est_ns = (walls[big_loops] - walls[1]) / (big_loops - 1) * 1e9
    return est_ns, walls


def _edges():
    single_senders = np.repeat(np.arange(K), K)
    single_receivers = np.tile(np.arange(K), K)
    batch_offset = np.arange(G)[:, None] * K
    c_senders = (single_senders[None, :] + batch_offset).reshape(-1).astype(np.int32)
    c_receivers = (single_receivers[None, :] + batch_offset).reshape(-1).astype(np.int32)
    c_edges = np.ones((c_senders.shape[0], 1), dtype=np.float32)
    return c_senders, c_receivers, c_edges


def _numpy_fallback(x, W1, b1, W2, b2, n_node):
    h = np.maximum(x @ W1 + b1, 0.0)
    logits = h @ W2 + b2
    lmax = logits.max(axis=-1, keepdims=True)
    ex = np.exp(logits - lmax)
    A = ex / ex.sum(axis=-1, keepdims=True)
    batch = np.repeat(np.arange(n_node.shape[0]), n_node)
    C = np.zeros((n_node.shape[0], K, x.shape[1]), dtype=np.float32)
    for k in range(K):
        np.add.at(C[:, k, :], batch, x * A[:, k:k + 1])
    return C.reshape(-1, x.shape[1]), A


def kernel(node_feats, W1, b1, W2, b2, n_node):
    from concourse import bass_utils

    x = np.ascontiguousarray(np.asarray(node_feats, dtype=np.float32))
    W1 = np.ascontiguousarray(np.asarray(W1, dtype=np.float32))
    b1v = np.asarray(b1, dtype=np.float32).reshape(-1)
    W2 = np.ascontiguousarray(np.asarray(W2, dtype=np.float32))
    b2v = np.asarray(b2, dtype=np.float32).reshape(-1)
    n_node = np.asarray(n_node, dtype=np.int32)

    c_senders, c_receivers, c_edges = _edges()

    if x.shape != (N, D) or not np.all(n_node == P):
        C, A = _numpy_fallback(x, W1, b1v, W2, b2v, n_node)
        return C, A, c_senders, c_receivers, c_edges

    nc = _get_program()

    b1c = b1v.reshape(H, 1).copy()
    b2r = np.tile(b2v.reshape(1, K), (128, 4)).copy()
    ident = np.eye(128, dtype=np.float32)

    in_maps = []
    for c in range(NCORES):
        in_maps.append({
            "x": x[c * NPC:(c + 1) * NPC],
            "w1": W1,
            "b1": b1c,
            "w2": W2,
            "b2r": b2r,
            "ident": ident,
        })

    res = bass_utils.run_bass_kernel_spmd(
        nc, in_maps, core_ids=list(range(NCORES)), trace=False
    )

    A = np.empty((N, K), dtype=np.float32)
    C = np.empty((G * K, D), dtype=np.float32)
    for c in range(NCORES):
        a_buf = res.results[c]["a_out"]              # [128, TILES, K]
        A[c * NPC:(c + 1) * NPC] = a_buf.transpose(1, 0, 2).reshape(NPC, K)
        c_buf = res.results[c]["c_out"].reshape(128, GPC, K)   # [D, g, k]
        C[c * GPC * K:(c + 1) * GPC * K] = (
            c_buf.transpose(1, 2, 0).reshape(GPC * K, D)
        )

    return C, A, c_senders, c_receivers, c_edges
